# revision 35
# baseline (speedup 1.0000x reference)
"""Bahdanau attention Trainium2 kernel (8-core SPMD, batch-parallel).

Math (per batch item b):
  proj  = features[b] @ W1 + W1_b + hidden[b] @ W2 + W2_b     (4096, 512)
  th    = tanh(proj)
  s     = th @ V_w            (V_b dropped: cancels exactly in softmax)
  w     = softmax(s)          (no max-subtraction: |s| <~ 1.5 here)
  ctx   = sum_f w[f] * features[b, f]
Outputs: (ctx [B,E] fp32, w [B,F,1] fp32)

Layout strategy per core (8 batch items, batch-pipelined via Tile bufs=2):
  - features loaded naturally [128F, 256E] fp32 (contiguous DMA, rotating
    window), cast to bf16 on GpSimd (keeps DVE free for PSUM drains)
  - per F-1024 chunk, just-in-time: PE-transpose to featT [E,F] bf16, then
    projT [U,F] = W1^T @ featT (bf16 in, fp32 PSUM accumulate over E)
  - tanh on ScalarE reading PSUM, fused per-partition bias
    (hidden@W2 + W1_b + W2_b, precomputed once), bf16 out
  - scores directly in [128F, 1] layout: tanh tile as bf16 stationary (fast
    weight load), V column as moving operand, PSUM-accumulated over U-chunks
    (avoids 16KB/partition single-partition score rows, any DVE score
    copies, and gives contiguous output DMA + direct ctx moving operand)
  - softmax without max-subtraction (|scores| <~ 1.5): exp + free-dim accum
    on ScalarE, partition-sum + 1/Z broadcast via tiny ones-matmuls,
    normalize on DVE -> w [128, 32]
  - ctx = sum over 32 F-chunks: matmul(w column stationary, bf16 features
    moving); attention weights PE-transposed to [32,128] for contiguous DMA
Engine budget (modeled, per core): PE ~195us stream + weight loads
(bottleneck, ~90% busy), ACT ~135us tanh/exp, DVE ~50us, GpSimd ~75us
casts, DMA ~93us (HBM floor). TimelineSim end-to-end: ~220us.
"""

import numpy as np

B, F, E, H, U = 64, 4096, 256, 512, 512
NCORES = 8
BPC = B // NCORES  # batches per core

_CACHE = {}
USE_XBAR = False


def _build():
    from contextlib import ExitStack

    import concourse.bass as bass
    import concourse.mybir as mybir
    import concourse.tile as tile
    from concourse import bacc
    from concourse.masks import make_identity

    f32 = mybir.dt.float32
    bf16 = mybir.dt.bfloat16
    AF = mybir.ActivationFunctionType
    OP = mybir.AluOpType
    AX = mybir.AxisListType

    nc = bacc.Bacc(
        "TRN2",
        target_bir_lowering=False,
        debug=False,
        enable_asserts=False,
        num_devices=NCORES,
    )

    feat_d = nc.dram_tensor("features", [BPC, F, E], f32, kind="ExternalInput").ap()
    hid_d = nc.dram_tensor("hidden", [BPC, H], f32, kind="ExternalInput").ap()
    w1_d = nc.dram_tensor("W1_w", [E, U], f32, kind="ExternalInput").ap()
    w1b_d = nc.dram_tensor("W1_b", [U], f32, kind="ExternalInput").ap()
    w2_d = nc.dram_tensor("W2_w", [H, U], f32, kind="ExternalInput").ap()
    w2b_d = nc.dram_tensor("W2_b", [U], f32, kind="ExternalInput").ap()
    v_d = nc.dram_tensor("V_w", [U, 1], f32, kind="ExternalInput").ap()
    ctx_d = nc.dram_tensor("ctx_out", [BPC, E], f32, kind="ExternalOutput").ap()
    attw_d = nc.dram_tensor("attw_out", [BPC, F], f32, kind="ExternalOutput").ap()

    NF = F // 128  # 32 f-chunks of 128
    NE = E // 128  # 2 e-chunks
    NU = U // 128  # 4 u-tiles
    NH = H // 128  # 4 h-chunks

    with tile.TileContext(nc) as tc, ExitStack() as ctx:
        # ---------------- pools ----------------
        consts = ctx.enter_context(tc.tile_pool(name="consts", bufs=1))
        wpool = ctx.enter_context(tc.tile_pool(name="weights", bufs=1))
        fnat_p = ctx.enter_context(tc.tile_pool(name="fnat", bufs=2))
        featT_p = ctx.enter_context(tc.tile_pool(name="featT", bufs=2))
        tanh_p = ctx.enter_context(tc.tile_pool(name="tanh", bufs=2))
        small_p = ctx.enter_context(tc.tile_pool(name="small", bufs=2))

        tr_ps = ctx.enter_context(tc.tile_pool(name="tr_ps", bufs=2, space="PSUM"))
        mm_ps = ctx.enter_context(tc.tile_pool(name="mm_ps", bufs=2, space="PSUM"))
        misc_ps = ctx.enter_context(tc.tile_pool(name="misc_ps", bufs=2, space="PSUM"))

        # ---------------- constants ----------------
        ident = consts.tile([128, 128], f32)
        make_identity(nc, ident[:])
        ident_bf = consts.tile([128, 128], bf16)
        nc.vector.tensor_copy(ident_bf[:], ident[:])
        ones_col = consts.tile([128, 1], f32)  # K=128 column of ones
        nc.gpsimd.memset(ones_col[:], 1.0)
        ones_row = consts.tile([1, 128], f32)  # K=1 row of ones
        nc.gpsimd.memset(ones_row[:], 1.0)

        # ---------------- weights ----------------
        w1_sb = []  # fp32 [128, 512] per e-chunk
        w1_bf = []  # bf16 casts
        for e in range(NE):
            w1t = wpool.tile([128, U], f32, name=f"w1_{e}")
            nc.sync.dma_start(w1t[:], w1_d[e * 128:(e + 1) * 128, :])
            w1b = wpool.tile([128, U], bf16, name=f"w1bf_{e}")
            nc.vector.tensor_copy(w1b[:], w1t[:])
            w1_sb.append(w1t)
            w1_bf.append(w1b)
        w2_sb = []
        for h in range(NH):
            w2t = wpool.tile([128, U], f32, name=f"w2_{h}")
            nc.sync.dma_start(w2t[:], w2_d[h * 128:(h + 1) * 128, :])
            w2_sb.append(w2t)

        # smalls block: rows 0-7 hidden, 8 W1_b, 9 W2_b, 10 V_w
        smalls = wpool.tile([11, H], f32)
        nc.sync.dma_start(smalls[0:BPC, :], hid_d[:, :])
        nc.sync.dma_start(smalls[8:9, :], w1b_d.rearrange("(o u) -> o u", o=1))
        nc.sync.dma_start(smalls[9:10, :], w2b_d.rearrange("(o u) -> o u", o=1))
        nc.sync.dma_start(smalls[10:11, :], v_d.rearrange("u o -> o u"))

        # transpose smalls 128-col chunks -> [128, 11] each
        smT = wpool.tile([128, 16 * NH], f32)  # chunk c at cols 16c..16c+10
        for c in range(NH):
            stp = misc_ps.tile([128, 16], f32, tag="m")
            nc.tensor.transpose(
                stp[:, 0:11], smalls[0:11, c * 128:(c + 1) * 128], ident[0:11, 0:11]
            )
            nc.vector.tensor_copy(smT[:, c * 16: c * 16 + 11], stp[:, 0:11])

        # V as bf16 stationary columns [128, NU]
        v_bf = wpool.tile([128, NU], bf16)
        for c in range(NU):
            nc.vector.tensor_copy(v_bf[:, c:c + 1], smT[:, c * 16 + 10: c * 16 + 11])

        # proj_h + biases: phb[u][:, b] = (hidden @ W2)[b, u-chunk] + W1_b + W2_b
        phb = wpool.tile([128, NU * BPC], f32)  # u-tile u at cols u*BPC..
        for u in range(NU):
            php = misc_ps.tile([128, BPC], f32, tag="m")
            for h in range(NH):
                nc.tensor.matmul(
                    php[:],
                    w2_sb[h][:, u * 128:(u + 1) * 128],
                    smT[:, h * 16: h * 16 + BPC],
                    start=(h == 0),
                    stop=(h == NH - 1),
                )
            bsum = wpool.tile([128, 1], f32, name=f"bsum_{u}")
            nc.vector.tensor_add(
                bsum[:], smT[:, u * 16 + 8: u * 16 + 9], smT[:, u * 16 + 9: u * 16 + 10]
            )
            nc.vector.tensor_scalar(
                phb[:, u * BPC:(u + 1) * BPC], php[:], bsum[:], None, op0=OP.add
            )

        # ---------------- main loop over batch items ----------------
        for b in range(BPC):
            # load natural features [128F, E] fp32 (rotating window), cast to
            # bf16 on gpsimd (idle engine); bf16 copies live the whole batch
            fbf = []
            for i in range(NF):
                fn = fnat_p.tile([128, E], f32, tag=f"fn{i % 16}", name=f"fn{i}_{b}")
                nc.sync.dma_start(fn[:], feat_d[b, i * 128:(i + 1) * 128, :])
                fb = fnat_p.tile([128, E], bf16, tag=f"fb{i}", name=f"fb{i}_{b}")
                nc.gpsimd.tensor_copy(fb[:], fn[:])
                fbf.append(fb)

            # transpose to featT bf16 [128E, F] (2 e-tiles), 8 blocks per psum buf
            featT = []
            for e in range(NE):
                ft = featT_p.tile([128, F], bf16, tag=f"ft{e}", name=f"ft{e}_{b}")
                featT.append(ft)
            # projT = W1^T @ featT (+bias) -> tanh bf16 [128U x NU, F]
            th = []
            for u in range(NU):
                t = tanh_p.tile([128, F], bf16, tag=f"th{u}", name=f"th{u}_{b}")
                th.append(t)
            mip = misc_ps.tile([128, 512], f32, tag="m", name=f"mi_{b}")
            for fc in range(F // 1024):
                # just-in-time transpose of this F-1024 chunk
                for e in range(NE):
                    trp = tr_ps.tile([128, 1024], bf16, tag="tr")
                    for j in range(8):
                        i = fc * 8 + j
                        nc.tensor.transpose(
                            trp[:, j * 128:(j + 1) * 128],
                            fbf[i][:, e * 128:(e + 1) * 128],
                            ident_bf[:],
                        )
                    nc.vector.tensor_copy(
                        featT[e][:, fc * 1024:(fc + 1) * 1024], trp[:]
                    )
                for u in range(NU):
                    mmp = mm_ps.tile([128, 1024], f32, tag="mm")
                    for e in range(NE):
                        for n in range(2):
                            nc.tensor.matmul(
                                mmp[:, n * 512:(n + 1) * 512],
                                w1_bf[e][:, u * 128:(u + 1) * 128],
                                featT[e][:, fc * 1024 + n * 512: fc * 1024 + (n + 1) * 512],
                                start=(e == 0),
                                stop=(e == NE - 1),
                            )
                    nc.scalar.activation(
                        th[u][:, fc * 1024:(fc + 1) * 1024],
                        mmp[:],
                        AF.Tanh,
                        bias=phb[:, u * BPC + b: u * BPC + b + 1],
                        scale=1.0,
                    )
                # scores for this F-1024 range, directly in [128F, 1] layout:
                # tanh tile as bf16 stationary (FWL), V column as moving
                for j in range(fc * 8, (fc + 1) * 8):
                    for u in range(NU):
                        nc.tensor.matmul(
                            mip[:, j:j + 1],
                            th[u][:, j * 128:(j + 1) * 128],
                            v_bf[:, u:u + 1],
                            start=(u == 0),
                            stop=(u == NU - 1),
                        )

            # exp + row-accumulate
            exp_sb = small_p.tile([128, NF], f32, tag="exp", name=f"exp_{b}")
            acc_sb = small_p.tile([128, 1], f32, tag="acc", name=f"acc_{b}")
            nc.scalar.activation(
                exp_sb[:], mip[:, 0:NF], AF.Exp, accum_out=acc_sb[:]
            )
            # Z = sum over partitions; rz = 1/Z broadcast to 128 partitions
            nc.tensor.matmul(
                mip[0:1, 64:65], ones_col[:], acc_sb[:], start=True, stop=True
            )
            z_sb = small_p.tile([1, 1], f32, tag="z", name=f"z_{b}")
            nc.vector.tensor_copy(z_sb[:], mip[0:1, 64:65])
            rz_sb = small_p.tile([1, 1], f32, tag="rz", name=f"rz_{b}")
            nc.vector.reciprocal(rz_sb[:], z_sb[:])
            nc.tensor.matmul(
                mip[:, 66:67], ones_row[:], rz_sb[0:1, 0:1], start=True, stop=True
            )
            rzb_sb = small_p.tile([128, 1], f32, tag="rzb", name=f"rzb_{b}")
            nc.vector.tensor_copy(rzb_sb[:], mip[:, 66:67])

            # w = exp * rz  [128, NF]
            w_sb = small_p.tile([128, NF], f32, tag="w", name=f"w_{b}")
            nc.vector.tensor_scalar(w_sb[:], exp_sb[:], rzb_sb[:], None, op0=OP.mult)
            w_bf = small_p.tile([128, NF], bf16, tag="wbf", name=f"wbf_{b}")
            nc.vector.tensor_copy(w_bf[:], w_sb[:])

            # attention weights out: transpose w -> [32, 128] then contiguous DMA
            nc.tensor.transpose(mip[0:32, 128:256], w_sb[:, 0:NF], ident[:])
            wt_out = small_p.tile([32, 128], f32, tag="wt_out", name=f"wto_{b}")
            nc.vector.tensor_copy(wt_out[:], mip[0:32, 128:256])
            nc.sync.dma_start(
                attw_d[b, :].rearrange("(a c) -> a c", a=32), wt_out[:]
            )

            # context: ctx[1, E] = sum_j w[:, j]^T @ fbf[j]  (bf16)
            for j in range(NF):
                nc.tensor.matmul(
                    mip[0:1, 256:512],
                    w_bf[:, j:j + 1],
                    fbf[j][:],
                    start=(j == 0),
                    stop=(j == NF - 1),
                )
            ctx_sb = small_p.tile([1, E], f32, tag="ctx", name=f"ctx_{b}")
            nc.vector.tensor_copy(ctx_sb[:], mip[0:1, 256:512])
            nc.sync.dma_start(ctx_d[b:b + 1, :], ctx_sb[:])

    nc.compile()
    return nc


def _get_nc():
    if "nc" not in _CACHE:
        _CACHE["nc"] = _build()
    return _CACHE["nc"]


def kernel(features, hidden, W1_w, W1_b, W2_w, W2_b, V_w, V_b):
    from concourse.bass_utils import run_bass_kernel_spmd

    nc = _get_nc()
    features = np.asarray(features, dtype=np.float32)
    hidden = np.asarray(hidden, dtype=np.float32)
    in_maps = []
    for c in range(NCORES):
        in_maps.append(
            {
                "features": np.ascontiguousarray(features[c * BPC:(c + 1) * BPC]),
                "hidden": np.ascontiguousarray(hidden[c * BPC:(c + 1) * BPC]),
                "W1_w": np.asarray(W1_w, dtype=np.float32),
                "W1_b": np.asarray(W1_b, dtype=np.float32),
                "W2_w": np.asarray(W2_w, dtype=np.float32),
                "W2_b": np.asarray(W2_b, dtype=np.float32),
                "V_w": np.asarray(V_w, dtype=np.float32),
            }
        )
    import os

    trace = bool(int(os.environ.get("KERNEL_TRACE", "0")))
    res = run_bass_kernel_spmd(
        nc, in_maps, core_ids=list(range(NCORES)), trace=trace
    )
    if trace:
        print(f"HW exec time: {res.exec_time_ns} ns")
        if res.instructions_and_trace is not None:
            print(f"trace: {res.instructions_and_trace[1]}")
        _CACHE["last_result"] = res
    ctxs = np.concatenate([r["ctx_out"] for r in res.results], axis=0)
    attw = np.concatenate([r["attw_out"] for r in res.results], axis=0)
    return ctxs, attw.reshape(B, F, 1)


# revision 40
# speedup vs baseline: 1.0055x; 1.0055x over previous
"""Bahdanau attention Trainium2 kernel (8-core SPMD, batch-parallel).

Math (per batch item b):
  proj  = features[b] @ W1 + W1_b + hidden[b] @ W2 + W2_b     (4096, 512)
  th    = tanh(proj)
  s     = th @ V_w            (V_b dropped: cancels exactly in softmax)
  w     = softmax(s)          (no max-subtraction: |s| <~ 1.5 here)
  ctx   = sum_f w[f] * features[b, f]
Outputs: (ctx [B,E] fp32, w [B,F,1] fp32)

Layout strategy per core (8 batch items, batch-pipelined via Tile bufs=2):
  - features loaded naturally [128F, 256E] fp32 (contiguous DMA, rotating
    window), cast to bf16 on GpSimd (keeps DVE free for PSUM drains)
  - per F-1024 chunk, just-in-time: PE-transpose to featT [E,F] bf16, then
    projT [U,F] = W1^T @ featT (bf16 in, fp32 PSUM accumulate over E)
  - tanh on ScalarE reading PSUM, fused per-partition bias
    (hidden@W2 + W1_b + W2_b, precomputed once), bf16 out
  - scores directly in [128F, 1] layout: tanh tile as bf16 stationary (fast
    weight load), V column as moving operand, PSUM-accumulated over U-chunks
    (avoids 16KB/partition single-partition score rows, any DVE score
    copies, and gives contiguous output DMA + direct ctx moving operand)
  - softmax without max-subtraction (|scores| <~ 1.5): exp + free-dim accum
    on ScalarE, partition-sum + 1/Z broadcast via tiny ones-matmuls,
    normalize on DVE -> w [128, 32]
  - ctx = sum over 32 F-chunks: matmul(w column stationary, bf16 features
    moving); attention weights PE-transposed to [32,128] for contiguous DMA
featT/tanh tiles are chunk-granular (bufs=3) and features have a
full-batch load-ahead window. PSUM hazard note: a start=True matmul zeroes
a region around its output, so each bank's accumulation group (ctx in mip)
must be the LAST group started in that bank.
Engine budget (modeled, per core): PE ~195us stream + weight loads
(bottleneck, ~90% busy), ACT ~135us tanh/exp, DVE ~50us, GpSimd ~75us
casts, DMA ~93us (HBM floor). TimelineSim end-to-end: ~219us.
"""

import numpy as np

B, F, E, H, U = 64, 4096, 256, 512, 512
NCORES = 8
BPC = B // NCORES  # batches per core

_CACHE = {}
USE_XBAR = False


def _build():
    from contextlib import ExitStack

    import concourse.bass as bass
    import concourse.mybir as mybir
    import concourse.tile as tile
    from concourse import bacc
    from concourse.masks import make_identity

    f32 = mybir.dt.float32
    bf16 = mybir.dt.bfloat16
    AF = mybir.ActivationFunctionType
    OP = mybir.AluOpType
    AX = mybir.AxisListType

    nc = bacc.Bacc(
        "TRN2",
        target_bir_lowering=False,
        debug=False,
        enable_asserts=False,
        num_devices=NCORES,
    )

    feat_d = nc.dram_tensor("features", [BPC, F, E], f32, kind="ExternalInput").ap()
    hid_d = nc.dram_tensor("hidden", [BPC, H], f32, kind="ExternalInput").ap()
    w1_d = nc.dram_tensor("W1_w", [E, U], f32, kind="ExternalInput").ap()
    w1b_d = nc.dram_tensor("W1_b", [U], f32, kind="ExternalInput").ap()
    w2_d = nc.dram_tensor("W2_w", [H, U], f32, kind="ExternalInput").ap()
    w2b_d = nc.dram_tensor("W2_b", [U], f32, kind="ExternalInput").ap()
    v_d = nc.dram_tensor("V_w", [U, 1], f32, kind="ExternalInput").ap()
    ctx_d = nc.dram_tensor("ctx_out", [BPC, E], f32, kind="ExternalOutput").ap()
    attw_d = nc.dram_tensor("attw_out", [BPC, F], f32, kind="ExternalOutput").ap()

    NF = F // 128  # 32 f-chunks of 128
    NE = E // 128  # 2 e-chunks
    NU = U // 128  # 4 u-tiles
    NH = H // 128  # 4 h-chunks

    with tile.TileContext(nc) as tc, ExitStack() as ctx:
        # ---------------- pools ----------------
        consts = ctx.enter_context(tc.tile_pool(name="consts", bufs=1))
        wpool = ctx.enter_context(tc.tile_pool(name="weights", bufs=1))
        fnat_p = ctx.enter_context(tc.tile_pool(name="fnat", bufs=2))
        featT_p = ctx.enter_context(tc.tile_pool(name="featT", bufs=3))
        tanh_p = ctx.enter_context(tc.tile_pool(name="tanh", bufs=3))
        small_p = ctx.enter_context(tc.tile_pool(name="small", bufs=2))

        tr_ps = ctx.enter_context(tc.tile_pool(name="tr_ps", bufs=2, space="PSUM"))
        mm_ps = ctx.enter_context(tc.tile_pool(name="mm_ps", bufs=2, space="PSUM"))
        misc_ps = ctx.enter_context(tc.tile_pool(name="misc_ps", bufs=2, space="PSUM"))

        # ---------------- constants ----------------
        ident = consts.tile([128, 128], f32)
        make_identity(nc, ident[:])
        ident_bf = consts.tile([128, 128], bf16)
        nc.vector.tensor_copy(ident_bf[:], ident[:])
        ones_col = consts.tile([128, 1], f32)  # K=128 column of ones
        nc.gpsimd.memset(ones_col[:], 1.0)
        ones_row = consts.tile([1, 128], f32)  # K=1 row of ones
        nc.gpsimd.memset(ones_row[:], 1.0)

        # ---------------- weights ----------------
        w1_sb = []  # fp32 [128, 512] per e-chunk
        w1_bf = []  # bf16 casts
        for e in range(NE):
            w1t = wpool.tile([128, U], f32, name=f"w1_{e}")
            nc.sync.dma_start(w1t[:], w1_d[e * 128:(e + 1) * 128, :])
            w1b = wpool.tile([128, U], bf16, name=f"w1bf_{e}")
            nc.vector.tensor_copy(w1b[:], w1t[:])
            w1_sb.append(w1t)
            w1_bf.append(w1b)
        w2_sb = []
        for h in range(NH):
            w2t = wpool.tile([128, U], f32, name=f"w2_{h}")
            nc.sync.dma_start(w2t[:], w2_d[h * 128:(h + 1) * 128, :])
            w2_sb.append(w2t)

        # smalls block: rows 0-7 hidden, 8 W1_b, 9 W2_b, 10 V_w
        smalls = wpool.tile([11, H], f32)
        nc.sync.dma_start(smalls[0:BPC, :], hid_d[:, :])
        nc.sync.dma_start(smalls[8:9, :], w1b_d.rearrange("(o u) -> o u", o=1))
        nc.sync.dma_start(smalls[9:10, :], w2b_d.rearrange("(o u) -> o u", o=1))
        nc.sync.dma_start(smalls[10:11, :], v_d.rearrange("u o -> o u"))

        # transpose smalls 128-col chunks -> [128, 11] each
        smT = wpool.tile([128, 16 * NH], f32)  # chunk c at cols 16c..16c+10
        for c in range(NH):
            stp = misc_ps.tile([128, 16], f32, tag="m")
            nc.tensor.transpose(
                stp[:, 0:11], smalls[0:11, c * 128:(c + 1) * 128], ident[0:11, 0:11]
            )
            nc.vector.tensor_copy(smT[:, c * 16: c * 16 + 11], stp[:, 0:11])

        # V as bf16 stationary columns [128, NU]
        v_bf = wpool.tile([128, NU], bf16)
        for c in range(NU):
            nc.vector.tensor_copy(v_bf[:, c:c + 1], smT[:, c * 16 + 10: c * 16 + 11])

        # proj_h + biases: phb[u][:, b] = (hidden @ W2)[b, u-chunk] + W1_b + W2_b
        phb = wpool.tile([128, NU * BPC], f32)  # u-tile u at cols u*BPC..
        for u in range(NU):
            php = misc_ps.tile([128, BPC], f32, tag="m")
            for h in range(NH):
                nc.tensor.matmul(
                    php[:],
                    w2_sb[h][:, u * 128:(u + 1) * 128],
                    smT[:, h * 16: h * 16 + BPC],
                    start=(h == 0),
                    stop=(h == NH - 1),
                )
            bsum = wpool.tile([128, 1], f32, name=f"bsum_{u}")
            nc.vector.tensor_add(
                bsum[:], smT[:, u * 16 + 8: u * 16 + 9], smT[:, u * 16 + 9: u * 16 + 10]
            )
            nc.vector.tensor_scalar(
                phb[:, u * BPC:(u + 1) * BPC], php[:], bsum[:], None, op0=OP.add
            )

        # ---------------- main loop over batch items ----------------
        for b in range(BPC):
            # load natural features [128F, E] fp32 (rotating window), cast to
            # bf16 on gpsimd (idle engine); bf16 copies live the whole batch
            fbf = []
            for i in range(NF):
                fn = fnat_p.tile([128, E], f32, tag=f"fn{i}", name=f"fn{i}_{b}")
                nc.sync.dma_start(fn[:], feat_d[b, i * 128:(i + 1) * 128, :])
                fb = fnat_p.tile([128, E], bf16, tag=f"fb{i}", name=f"fb{i}_{b}")
                nc.gpsimd.tensor_copy(fb[:], fn[:])
                fbf.append(fb)

            # per F-1024 chunk: JIT transpose -> mm1 -> tanh -> scores, with
            # chunk-granular featT/tanh tiles (bufs=3) for deep pipelining
            mip = misc_ps.tile([128, 512], f32, tag="m", name=f"mi_{b}")
            for fc in range(F // 1024):
                ftc = []
                for e in range(NE):
                    ft = featT_p.tile(
                        [128, 1024], bf16, tag=f"ft{e}", name=f"ft{e}_{b}_{fc}"
                    )
                    ftc.append(ft)
                    trp = tr_ps.tile([128, 1024], bf16, tag="tr")
                    for j in range(8):
                        i = fc * 8 + j
                        nc.tensor.transpose(
                            trp[:, j * 128:(j + 1) * 128],
                            fbf[i][:, e * 128:(e + 1) * 128],
                            ident_bf[:],
                        )
                    nc.vector.tensor_copy(ft[:], trp[:])
                thc = []
                for u in range(NU):
                    t = tanh_p.tile(
                        [128, 1024], bf16, tag=f"th{u}", name=f"th{u}_{b}_{fc}"
                    )
                    thc.append(t)
                    mmp = mm_ps.tile([128, 1024], f32, tag="mm")
                    for e in range(NE):
                        for n in range(2):
                            nc.tensor.matmul(
                                mmp[:, n * 512:(n + 1) * 512],
                                w1_bf[e][:, u * 128:(u + 1) * 128],
                                ftc[e][:, n * 512:(n + 1) * 512],
                                start=(e == 0),
                                stop=(e == NE - 1),
                            )
                    nc.scalar.activation(
                        t[:],
                        mmp[:],
                        AF.Tanh,
                        bias=phb[:, u * BPC + b: u * BPC + b + 1],
                        scale=1.0,
                    )
                # scores for this F-1024 range, directly in [128F, 1] layout:
                # tanh tile as bf16 stationary (FWL), V column as moving
                for jl in range(8):
                    j = fc * 8 + jl
                    for u in range(NU):
                        nc.tensor.matmul(
                            mip[:, j:j + 1],
                            thc[u][:, jl * 128:(jl + 1) * 128],
                            v_bf[:, u:u + 1],
                            start=(u == 0),
                            stop=(u == NU - 1),
                        )

            # exp + row-accumulate
            exp_sb = small_p.tile([128, NF], f32, tag="exp", name=f"exp_{b}")
            acc_sb = small_p.tile([128, 1], f32, tag="acc", name=f"acc_{b}")
            nc.scalar.activation(
                exp_sb[:], mip[:, 0:NF], AF.Exp, accum_out=acc_sb[:]
            )
            # Z = sum over partitions; rz = 1/Z broadcast to 128 partitions
            nc.tensor.matmul(
                mip[0:1, 64:65], ones_col[:], acc_sb[:], start=True, stop=True
            )
            z_sb = small_p.tile([1, 1], f32, tag="z", name=f"z_{b}")
            nc.vector.tensor_copy(z_sb[:], mip[0:1, 64:65])
            rz_sb = small_p.tile([1, 1], f32, tag="rz", name=f"rz_{b}")
            nc.vector.reciprocal(rz_sb[:], z_sb[:])
            nc.tensor.matmul(
                mip[:, 66:67], ones_row[:], rz_sb[0:1, 0:1], start=True, stop=True
            )
            rzb_sb = small_p.tile([128, 1], f32, tag="rzb", name=f"rzb_{b}")
            nc.vector.tensor_copy(rzb_sb[:], mip[:, 66:67])

            # w = exp * rz  [128, NF]
            w_sb = small_p.tile([128, NF], f32, tag="w", name=f"w_{b}")
            nc.vector.tensor_scalar(w_sb[:], exp_sb[:], rzb_sb[:], None, op0=OP.mult)
            w_bf = small_p.tile([128, NF], bf16, tag="wbf", name=f"wbf_{b}")
            nc.vector.tensor_copy(w_bf[:], w_sb[:])

            # attention weights out: transpose w -> [32, 128] then contiguous DMA
            nc.tensor.transpose(mip[0:32, 128:256], w_sb[:, 0:NF], ident[:])
            wt_out = small_p.tile([32, 128], f32, tag="wt_out", name=f"wto_{b}")
            nc.vector.tensor_copy(wt_out[:], mip[0:32, 128:256])
            nc.sync.dma_start(
                attw_d[b, :].rearrange("(a c) -> a c", a=32), wt_out[:]
            )

            # context: ctx[1, E] = sum_j w[:, j]^T @ fbf[j]  (bf16)
            for j in range(NF):
                nc.tensor.matmul(
                    mip[0:1, 256:512],
                    w_bf[:, j:j + 1],
                    fbf[j][:],
                    start=(j == 0),
                    stop=(j == NF - 1),
                )
            ctx_sb = small_p.tile([1, E], f32, tag="ctx", name=f"ctx_{b}")
            nc.vector.tensor_copy(ctx_sb[:], mip[0:1, 256:512])
            nc.sync.dma_start(ctx_d[b:b + 1, :], ctx_sb[:])

    nc.compile()
    return nc


def _get_nc():
    if "nc" not in _CACHE:
        _CACHE["nc"] = _build()
    return _CACHE["nc"]


def kernel(features, hidden, W1_w, W1_b, W2_w, W2_b, V_w, V_b):
    from concourse.bass_utils import run_bass_kernel_spmd

    nc = _get_nc()
    features = np.asarray(features, dtype=np.float32)
    hidden = np.asarray(hidden, dtype=np.float32)
    in_maps = []
    for c in range(NCORES):
        in_maps.append(
            {
                "features": np.ascontiguousarray(features[c * BPC:(c + 1) * BPC]),
                "hidden": np.ascontiguousarray(hidden[c * BPC:(c + 1) * BPC]),
                "W1_w": np.asarray(W1_w, dtype=np.float32),
                "W1_b": np.asarray(W1_b, dtype=np.float32),
                "W2_w": np.asarray(W2_w, dtype=np.float32),
                "W2_b": np.asarray(W2_b, dtype=np.float32),
                "V_w": np.asarray(V_w, dtype=np.float32),
            }
        )
    import os

    trace = bool(int(os.environ.get("KERNEL_TRACE", "0")))
    res = run_bass_kernel_spmd(
        nc, in_maps, core_ids=list(range(NCORES)), trace=trace
    )
    if trace:
        print(f"HW exec time: {res.exec_time_ns} ns")
        if res.instructions_and_trace is not None:
            print(f"trace: {res.instructions_and_trace[1]}")
        _CACHE["last_result"] = res
    ctxs = np.concatenate([r["ctx_out"] for r in res.results], axis=0)
    attw = np.concatenate([r["attw_out"] for r in res.results], axis=0)
    return ctxs, attw.reshape(B, F, 1)
